# revision 22
# baseline (speedup 1.0000x reference)
"""Trainium2 Bass kernel for nn_AttentionBlock (GroupNorm + 8-head self-attention).

Data-parallel over batch: 8 batch elements -> 8 NeuronCores, one each.

V2a layout ([c, n] with c on partitions, c = 4 chunks x 128, n = 1024):
  GroupNorm   : bn_stats per channel -> group-combine matmul (G) ->
                broadcast matmul (G^T) -> fused scale/shift -> xn (bf16)
  QKV         : Q,K in [o, n] layout (lhsT = wqkvT chunks, bf16 from host),
                V in [n, o] layout (V^T) with a 65th ones-column per head
  Attention   : per head-pair p, per nu half (512 queries), 8 m-chunk steps:
                  S^T = K^T Q    (bf16, both heads packed in [128,1024] PSUM)
                  P = exp(S/8)   (ScalarE, PSUM->SBUF bf16)
                  AV_e[0:65,:] += VT65_e^T @ P_e  (M=65: row 64 = rowsum)
                drain: rowsum rows ->DMA-> recip (bf16) ->DMA roundtrip
                broadcast [128,1024]; AV ->DMA-> att; one in-place
                scalar_tensor_tensor att *= R
  Proj        : fp32r matmul + residual add, output fp32
"""

import numpy as np

NUM_GROUPS = 32
NUM_HEADS = 8
EPS = 1e-6
C = 512
N = 1024
B = 8

_cache = {}


def _build_bass():
    import concourse.bacc as bacc
    import concourse.bass as bass_mod
    import concourse.mybir as mybir
    import concourse.tile as tile

    fp32 = mybir.dt.float32
    fp32r = mybir.dt.float32r
    bf16 = mybir.dt.bfloat16
    fp8 = mybir.dt.float8e4
    u8 = mybir.dt.uint8
    DR = mybir.MatmulPerfMode.DoubleRow
    AF = mybir.ActivationFunctionType
    OP = mybir.AluOpType

    nc = bacc.Bacc("TRN2", target_bir_lowering=False, debug=False)

    x_d = nc.dram_tensor("x", [C, N], fp32, kind="ExternalInput")
    wqkvT_d = nc.dram_tensor("wqkvT", [C, 3 * C], bf16, kind="ExternalInput")
    projT_d = nc.dram_tensor("projT", [C, C], fp32, kind="ExternalInput")
    qkvb_d = nc.dram_tensor("qkv_b", [3 * C], fp32, kind="ExternalInput")
    projb_d = nc.dram_tensor("proj_b", [C], fp32, kind="ExternalInput")
    nw_d = nc.dram_tensor("norm_w", [C], fp32, kind="ExternalInput")
    nb_d = nc.dram_tensor("norm_b", [C], fp32, kind="ExternalInput")
    G_d = nc.dram_tensor("Gmat", [128, 4, 32], fp32, kind="ExternalInput")
    GT_d = nc.dram_tensor("GTmat", [32, 4, 128], fp32, kind="ExternalInput")
    y_d = nc.dram_tensor("y", [C, N], fp32, kind="ExternalOutput")
    r8r_d = nc.dram_tensor("r8r_scratch", [4, 2, 2, 512], fp32)

    with tile.TileContext(nc) as tc:
        with (
            tc.tile_pool(name="const", bufs=1) as const,
            tc.tile_pool(name="work", bufs=1) as work,
            tc.tile_pool(name="ppool", bufs=2) as ppool,
            tc.tile_pool(name="rpool", bufs=3) as rpool,
            tc.tile_pool(name="pss", bufs=3, space="PSUM") as pss,     # 3x[128,1024]
            tc.tile_pool(name="psav", bufs=2, space="PSUM") as psav,   # 2x[128,512]
        ):
            # ---------------- load inputs ----------------
            x_sb = work.tile([128, 4, N], fp32, tag="x")
            _dma_engines = [nc.sync, nc.scalar, nc.gpsimd, nc.sync]
            _x8 = x_d.ap().rearrange("(j p) (u n) -> j u p n", p=128, n=512)
            for j in range(4):
                for u in range(2):
                    [nc.sync, nc.scalar, nc.gpsimd][(2 * j + u) % 3].dma_start(
                        x_sb[:, j, u * 512 : u * 512 + 512], _x8[j, u]
                    )

            w_bf = work.tile([128, 4, 3 * C], bf16, tag="wbf")
            for j in range(4):
                [nc.scalar, nc.sync, nc.scalar, nc.sync][j].dma_start(
                    w_bf[:, j, :], wqkvT_d.ap().rearrange("(j p) o -> j p o", p=128)[j]
                )
            p_r = work.tile([128, 4, C], fp32r, tag="pr")
            for j in range(4):
                stg = rpool.tile([128, C], fp32, tag="Rb", name="pstage")
                nc.sync.dma_start(
                    stg[:], projT_d.ap().rearrange("(j p) o -> j p o", p=128)[j]
                )
                nc.scalar.copy(p_r[:, j, :], stg[:])

            G_sb = const.tile([128, 4, 32], fp32, tag="G")
            GT_sb = const.tile([32, 4, 128], fp32, tag="GT")
            nc.sync.dma_start(G_sb[:], G_d.ap())
            nc.sync.dma_start(GT_sb[:], GT_d.ap())
            nw_sb = const.tile([128, 4], fp32, tag="nw")
            nb_sb = const.tile([128, 4], fp32, tag="nb")
            nc.sync.dma_start(nw_sb[:], nw_d.ap().rearrange("(j p) -> p j", p=128))
            nc.sync.dma_start(nb_sb[:], nb_d.ap().rearrange("(j p) -> p j", p=128))
            qb_sb = const.tile([128, 8], fp32, tag="qb")
            nc.sync.dma_start(
                qb_sb[:], qkvb_d.ap()[0 : 2 * C].rearrange("(o p) -> p o", p=128)
            )
            pb_sb = const.tile([128, 4], fp32, tag="pb")
            nc.sync.dma_start(pb_sb[:], projb_d.ap().rearrange("(j p) -> p j", p=128))
            vb_src = qkvb_d.ap()[2 * C : 3 * C]
            vb_bcast_ap = bass_mod.AP(
                tensor=vb_src.tensor, offset=vb_src.offset, ap=[[0, 128], [1, C]]
            )
            vb_bc = const.tile([128, C], fp32, tag="vbbc")
            nc.sync.dma_start(vb_bc[:], vb_bcast_ap)
            mc_sb = const.tile([128, 1], fp32, tag="mc")
            nc.vector.memset(mc_sb[:], -2.5)
            # pre-warm the exp/ln ACT table set during input DMA
            warm = const.tile([32, 1], fp32, tag="warm")
            nc.vector.memset(warm[:], 1.0)
            nc.scalar.activation(warm[:], warm[:], AF.Exp, scale=1.0)

            # ---------------- groupnorm ----------------
            stats = work.tile([128, 4, 2, 6], fp32, tag="stats")
            for j in range(4):
                for u in range(2):
                    nc.vector.bn_stats(
                        stats[:, j, u, :], x_sb[:, j, u * 512 : u * 512 + 512]
                    )
            mv = work.tile([128, 4, 2], fp32, tag="mv")
            for j in range(4):
                nc.vector.bn_aggr(mv[:, j, :], stats[:, j, :, :])
            ssq = work.tile([128, 4, 2], fp32, tag="ssq")
            nc.vector.tensor_copy(ssq[:, :, 0], mv[:, :, 0])
            nc.vector.tensor_tensor(ssq[:, :, 1], mv[:, :, 0], mv[:, :, 0], op=OP.mult)
            nc.vector.tensor_tensor(ssq[:, :, 1], ssq[:, :, 1], mv[:, :, 1], op=OP.add)
            # group stats [32, 2] = (mu_g, E[x^2]_g); G has 1/16 entries
            ps_g = psav.tile([32, 2], fp32, tag="av")
            for j in range(4):
                nc.tensor.matmul(
                    ps_g[:], G_sb[:, j, :], ssq[:, j, :], start=(j == 0), stop=(j == 3)
                )
            st2 = work.tile([32, 2], fp32, tag="st2")
            nc.vector.tensor_copy(st2[:, 0:1], ps_g[:, 0:1])
            var = work.tile([32, 1], fp32, tag="var")
            nc.vector.tensor_tensor(var[:], st2[:, 0:1], st2[:, 0:1], op=OP.mult)
            nc.vector.tensor_tensor(var[:], ps_g[:, 1:2], var[:], op=OP.subtract)
            eps_sb = const.tile([32, 1], fp32, tag="eps")
            nc.vector.memset(eps_sb[:], float(EPS))
            nc.scalar.activation(var[:], var[:], AF.Ln, bias=eps_sb[:], scale=1.0)
            nc.scalar.activation(st2[:, 1:2], var[:], AF.Exp, scale=-0.5)
            ps_bc = psav.tile([128, 4, 2], fp32, tag="av")
            for j in range(4):
                nc.tensor.matmul(
                    ps_bc[:, j, :], GT_sb[:, j, :], st2[:], start=True, stop=True
                )
            ab = work.tile([128, 4, 2], fp32, tag="ab")
            nc.vector.tensor_tensor(ab[:, :, 0], ps_bc[:, :, 1], nw_sb[:], op=OP.mult)
            nc.vector.tensor_tensor(ab[:, :, 1], ps_bc[:, :, 0], ab[:, :, 0], op=OP.mult)
            nc.vector.tensor_tensor(ab[:, :, 1], nb_sb[:], ab[:, :, 1], op=OP.subtract)

            xn_bf = work.tile([128, 4, N], bf16, tag="xn")
            for j in range(4):
                nc.gpsimd.tensor_scalar(
                    xn_bf[:, j, :],
                    x_sb[:, j, :],
                    ab[:, j, 0:1],
                    ab[:, j, 1:2],
                    op0=OP.mult,
                    op1=OP.add,
                )

            # ---------------- V^T = xn^T @ wV^T : [n, head, 65] ----------------
            # column 64 of each head's block is all-ones -> rowsum in AV row 64
            VT65 = work.tile([128, 8, 8, 66], bf16, tag="VT65")
            nc.vector.memset(VT65[:, :, :, 64:66], 1.0)

            def emit_v_chunks(mcs):
                for mc in mcs:
                    ps_v = psav.tile([128, 512], fp32, tag="av", name="ps_v")
                    for k in range(4):
                        nc.tensor.matmul(
                            ps_v[:],
                            xn_bf[:, k, mc * 128 : mc * 128 + 128],
                            w_bf[:, k, 2 * C : 3 * C],
                            start=(k == 0),
                            stop=(k == 3),
                        )
                    nc.vector.tensor_tensor(
                        VT65[:, mc, :, 0:64],
                        ps_v[:].rearrange("q (h d) -> q h d", h=8),
                        vb_bc[:].rearrange("q (h d) -> q h d", h=8),
                        op=OP.add,
                    )

            # ---------------- Q, K chunks (emitted interleaved) ----------------
            QK_bf = work.tile([128, 8, N], bf16, tag="QK")

            def emit_qk(oc):
                ps_qk = pss.tile([128, 1024], fp32, tag="s")
                for nu in range(2):
                    for k in range(4):
                        nc.tensor.matmul(
                            ps_qk[:, nu * 512 : nu * 512 + 512],
                            w_bf[:, k, oc * 128 : oc * 128 + 128],
                            xn_bf[:, k, nu * 512 : nu * 512 + 512],
                            start=(k == 0),
                            stop=(k == 3),
                        )
                nc.vector.tensor_scalar(
                    QK_bf[:, oc, :], ps_qk[:], qb_sb[:, oc : oc + 1], None, op0=OP.add
                )

            def emit_qk_small(oc):
                # mid-attention QK: psav bank, two half generations
                for nu in range(2):
                    ps_qk = psav.tile([128, 512], fp32, tag="av", name="ps_qk")
                    for k in range(4):
                        nc.tensor.matmul(
                            ps_qk[:],
                            w_bf[:, k, oc * 128 : oc * 128 + 128],
                            xn_bf[:, k, nu * 512 : nu * 512 + 512],
                            start=(k == 0),
                            stop=(k == 3),
                        )
                    nc.vector.tensor_scalar(
                        QK_bf[:, oc, nu * 512 : nu * 512 + 512],
                        ps_qk[:],
                        qb_sb[:, oc : oc + 1],
                        None,
                        op0=OP.add,
                    )

            att = work.tile([128, 4, N], fp32, tag="att")
            att_r = att.bitcast(fp32r)

            def s_mm(units, p, nu, s):
                for e in range(2):
                    nc.tensor.matmul(
                        units[:, e * 512 : e * 512 + 512],
                        QK_bf[e * 64 : e * 64 + 64, 4 + p, s * 128 : s * 128 + 128],
                        QK_bf[e * 64 : e * 64 + 64, p, nu * 512 : nu * 512 + 512],
                        start=True,
                        stop=True,
                        skip_group_check=True,
                    )

            def emit_pass(p, nu, fillers={}):
                """One nu-half of head-pair p: 8 s-steps of S -> exp -> AV."""
                av = [
                    psav.tile([128, 512], fp32, tag="av", name=f"av{e}")
                    for e in range(2)
                ]
                units = [None, None, None]
                units[0] = pss.tile([128, 1024], fp32, tag="s", name="ps_s")
                s_mm(units[0], p, nu, 0)
                units[1] = pss.tile([128, 1024], fp32, tag="s", name="ps_s")
                s_mm(units[1], p, nu, 1)
                for s in range(8):
                    P_bf = ppool.tile([128, 2, 512], bf16, tag="P")
                    nc.scalar.activation(
                        P_bf[:],
                        units[s % 3][:].rearrange("q (h n) -> q h n", h=2),
                        AF.Exp,
                        bias=mc_sb[:],
                        scale=0.125,
                    )
                    if s < 6:
                        units[(s + 2) % 3] = pss.tile(
                            [128, 1024], fp32, tag="s", name="ps_s"
                        )
                        s_mm(units[(s + 2) % 3], p, nu, s + 2)
                    for e in range(2):
                        nc.tensor.matmul(
                            av[e][0:65, :],
                            VT65[:, s, 2 * p + e, 0:65],
                            P_bf[:, e, :],
                            start=(s == 0),
                            stop=(s == 7),
                            skip_group_check=True,
                        )
                    if s in fillers:
                        fillers[s]()
                # rowsum rows -> partition 0, recip there, DMA roundtrip bcast
                rr_st = rpool.tile([128, 2, 512], fp32, tag="Rb", name="rrst")
                nc.scalar.copy(rr_st[0:1, 0, :], av[0][64:65, :])
                nc.vector.tensor_copy(rr_st[0:1, 1, :], av[1][64:65, :])
                rr_raw = rpool.tile([128, 2, 512], fp32, tag="Rb", name="rraw")
                nc.vector.reciprocal_approx_fast(
                    rr_raw[0:1, :, :].rearrange("q e n -> q (e n)"),
                    rr_st[0:1, :, :].rearrange("q e n -> q (e n)"),
                )
                nc.sync.dma_start(r8r_d.ap()[p, nu], rr_raw[0:1, :, :])
                for e in range(2):
                    nc.vector.tensor_copy(
                        att_r[e * 64 : e * 64 + 64, p, nu * 512 : nu * 512 + 512],
                        av[e][0:64, :],
                    )
                Rb = rpool.tile([128, 2, 512], fp32, tag="Rb", name="Rb")
                for e, eng in ((0, nc.sync), (1, nc.scalar)):
                    src_ap = r8r_d.ap()[p, nu, e]
                    bcast = bass_mod.AP(
                        tensor=src_ap.tensor,
                        offset=src_ap.offset,
                        ap=[[0, 64], [1, 512]],
                    )
                    eng.dma_start(Rb[e * 64 : e * 64 + 64, 0, :], bcast)
                nc.gpsimd.tensor_tensor(
                    att_r[:, p, nu * 512 : nu * 512 + 512],
                    att[:, p, nu * 512 : nu * 512 + 512],
                    Rb[:, 0, :],
                    op=OP.mult,
                )

            emit_qk(0)
            emit_qk(4)
            emit_v_chunks([0, 1, 2, 3, 4, 5, 6, 7])
            emit_pass(0, 0)
            emit_qk_small(1)
            emit_pass(0, 1)
            emit_qk_small(5)
            emit_pass(1, 0)
            emit_qk_small(2)
            emit_pass(1, 1)
            emit_qk_small(6)
            emit_pass(2, 0)
            emit_qk_small(3)
            emit_pass(2, 1)
            emit_qk_small(7)
            emit_pass(3, 0)
            emit_pass(3, 1)

            # ---------------- proj + residual ----------------
            for j in range(4):
                nc.vector.tensor_scalar(
                    x_sb[:, j, :], x_sb[:, j, :], pb_sb[:, j : j + 1], None, op0=OP.add
                )
            for oc in range(4):
                ps_o = pss.tile([128, 1024], fp32, tag="s")
                for nu in range(2):
                    for k in range(4):
                        nc.tensor.matmul(
                            ps_o[:, nu * 512 : nu * 512 + 512],
                            p_r[:, k, oc * 128 : oc * 128 + 128],
                            att_r[:, k, nu * 512 : nu * 512 + 512],
                            start=(k == 0),
                            stop=(k == 3),
                        )
                nc.vector.tensor_tensor(
                    x_sb[:, oc, :], ps_o[:], x_sb[:, oc, :], op=OP.add
                )
                _y8 = y_d.ap().rearrange("(j p) (u n) -> j u p n", p=128, n=512)
                for u in range(2):
                    [nc.sync, nc.scalar][u].dma_start(
                        _y8[oc, u], x_sb[:, oc, u * 512 : u * 512 + 512]
                    )

    nc.compile()
    return nc


def _get_nc(debug=False):
    if "nc" not in _cache:
        _cache["nc"] = _build_bass()
    return _cache["nc"]


def _host_inputs(x, norm_w, norm_b, qkv_w, qkv_b, proj_w, proj_b):
    import ml_dtypes

    x = np.asarray(x, dtype=np.float32).reshape(B, C, N)
    wqkvT = np.ascontiguousarray(
        np.asarray(qkv_w, dtype=np.float32).T.astype(ml_dtypes.bfloat16)
    )
    projT = np.ascontiguousarray(np.asarray(proj_w, dtype=np.float32).T)
    G = np.zeros((128, 4, 32), dtype=np.float32)
    GT = np.zeros((32, 4, 128), dtype=np.float32)
    for j in range(4):
        for p in range(128):
            g = 8 * j + p // 16
            G[p, j, g] = 1.0 / 16.0
            GT[g, j, p] = 1.0
    shared = {
        "wqkvT": wqkvT,
        "projT": projT,
        "qkv_b": np.asarray(qkv_b, dtype=np.float32),
        "proj_b": np.asarray(proj_b, dtype=np.float32),
        "norm_w": np.asarray(norm_w, dtype=np.float32),
        "norm_b": np.asarray(norm_b, dtype=np.float32),
        "Gmat": G,
        "GTmat": GT,
    }
    in_maps = [dict(shared, x=np.ascontiguousarray(x[i])) for i in range(B)]
    return in_maps


def kernel(x, norm_w, norm_b, qkv_w, qkv_b, proj_w, proj_b, _trace=False):
    from concourse import bass_utils

    nc = _get_nc()
    in_maps = _host_inputs(x, norm_w, norm_b, qkv_w, qkv_b, proj_w, proj_b)
    res = bass_utils.run_bass_kernel_spmd(
        nc, in_maps, core_ids=list(range(B)), trace=_trace
    )
    out = np.stack([res.results[i]["y"] for i in range(B)])
    _cache["last_result"] = res
    return out.reshape(B, C, 32, 32)


# revision 23
# speedup vs baseline: 1.1476x; 1.1476x over previous
"""Trainium2 Bass kernel for nn_AttentionBlock (GroupNorm + 8-head self-attention).

Data-parallel over batch: 8 batch elements -> 8 NeuronCores, one each.

V2a layout ([c, n] with c on partitions, c = 4 chunks x 128, n = 1024):
  GroupNorm   : bn_stats per channel -> group-combine matmul (G) ->
                broadcast matmul (G^T) -> fused scale/shift -> xn (bf16)
  QKV         : Q,K in [o, n] layout (lhsT = wqkvT chunks, bf16 from host),
                V in [n, o] layout (V^T) with a 65th ones-column per head
  Attention   : per head-pair p, per nu half (512 queries), 8 m-chunk steps:
                  S^T = K^T Q    (bf16, both heads packed in [128,1024] PSUM)
                  P = exp(S/8)   (ScalarE, PSUM->SBUF bf16)
                  AV_e[0:65,:] += VT65_e^T @ P_e  (M=65: row 64 = rowsum)
                drain: rowsum rows ->DMA-> recip (bf16) ->DMA roundtrip
                broadcast [128,1024]; AV ->DMA-> att; one in-place
                scalar_tensor_tensor att *= R
  Proj        : fp32r matmul + residual add, output fp32
"""

import numpy as np

NUM_GROUPS = 32
NUM_HEADS = 8
EPS = 1e-6
C = 512
N = 1024
B = 8

_cache = {}


def _build_bass():
    import concourse.bacc as bacc
    import concourse.bass as bass_mod
    import concourse.mybir as mybir
    import concourse.tile as tile

    fp32 = mybir.dt.float32
    fp32r = mybir.dt.float32r
    bf16 = mybir.dt.bfloat16
    fp8 = mybir.dt.float8e4
    u8 = mybir.dt.uint8
    DR = mybir.MatmulPerfMode.DoubleRow
    AF = mybir.ActivationFunctionType
    OP = mybir.AluOpType

    nc = bacc.Bacc("TRN2", target_bir_lowering=False, debug=False)

    x_d = nc.dram_tensor("x", [C, N], fp32, kind="ExternalInput")
    wqkvT_d = nc.dram_tensor("wqkvT", [C, 3 * C], bf16, kind="ExternalInput")
    projT_d = nc.dram_tensor("projT", [C, C], fp32, kind="ExternalInput")
    qkvb_d = nc.dram_tensor("qkv_b", [3 * C], fp32, kind="ExternalInput")
    projb_d = nc.dram_tensor("proj_b", [C], fp32, kind="ExternalInput")
    nw_d = nc.dram_tensor("norm_w", [C], fp32, kind="ExternalInput")
    nb_d = nc.dram_tensor("norm_b", [C], fp32, kind="ExternalInput")
    G_d = nc.dram_tensor("Gmat", [128, 4, 32], fp32, kind="ExternalInput")
    GT_d = nc.dram_tensor("GTmat", [32, 4, 128], fp32, kind="ExternalInput")
    y_d = nc.dram_tensor("y", [C, N], fp32, kind="ExternalOutput")
    r8r_d = nc.dram_tensor("r8r_scratch", [4, 2, 2, 512], fp32)

    with tile.TileContext(nc) as tc:
        with (
            tc.tile_pool(name="const", bufs=1) as const,
            tc.tile_pool(name="work", bufs=1) as work,
            tc.tile_pool(name="ppool", bufs=2) as ppool,
            tc.tile_pool(name="rpool", bufs=3) as rpool,
            tc.tile_pool(name="pss", bufs=2, space="PSUM") as pss,     # 2x[128,1024]
            tc.tile_pool(name="psav", bufs=3, space="PSUM") as psav,   # 3x[128,512]
            tc.tile_pool(name="psq", bufs=1, space="PSUM") as psq,     # 1x[128,512]
        ):
            # ---------------- load inputs ----------------
            x_sb = work.tile([128, 4, N], fp32, tag="x")
            _dma_engines = [nc.sync, nc.scalar, nc.gpsimd, nc.sync]
            _x8 = x_d.ap().rearrange("(j p) (u n) -> j u p n", p=128, n=512)
            for j in range(4):
                for u in range(2):
                    [nc.sync, nc.scalar, nc.gpsimd][(2 * j + u) % 3].dma_start(
                        x_sb[:, j, u * 512 : u * 512 + 512], _x8[j, u]
                    )

            w_bf = work.tile([128, 4, 3 * C], bf16, tag="wbf")
            for j in range(4):
                [nc.scalar, nc.sync, nc.scalar, nc.sync][j].dma_start(
                    w_bf[:, j, :], wqkvT_d.ap().rearrange("(j p) o -> j p o", p=128)[j]
                )
            p_r = work.tile([128, 4, C], fp32r, tag="pr")
            for j in range(4):
                stg = rpool.tile([128, C], fp32, tag="Rb", name="pstage")
                nc.sync.dma_start(
                    stg[:], projT_d.ap().rearrange("(j p) o -> j p o", p=128)[j]
                )
                nc.scalar.copy(p_r[:, j, :], stg[:])

            G_sb = const.tile([128, 4, 32], fp32, tag="G")
            GT_sb = const.tile([32, 4, 128], fp32, tag="GT")
            nc.sync.dma_start(G_sb[:], G_d.ap())
            nc.sync.dma_start(GT_sb[:], GT_d.ap())
            nw_sb = const.tile([128, 4], fp32, tag="nw")
            nb_sb = const.tile([128, 4], fp32, tag="nb")
            nc.sync.dma_start(nw_sb[:], nw_d.ap().rearrange("(j p) -> p j", p=128))
            nc.sync.dma_start(nb_sb[:], nb_d.ap().rearrange("(j p) -> p j", p=128))
            qb_sb = const.tile([128, 8], fp32, tag="qb")
            nc.sync.dma_start(
                qb_sb[:], qkvb_d.ap()[0 : 2 * C].rearrange("(o p) -> p o", p=128)
            )
            pb_sb = const.tile([128, 4], fp32, tag="pb")
            nc.sync.dma_start(pb_sb[:], projb_d.ap().rearrange("(j p) -> p j", p=128))
            vb_src = qkvb_d.ap()[2 * C : 3 * C]
            vb_bcast_ap = bass_mod.AP(
                tensor=vb_src.tensor, offset=vb_src.offset, ap=[[0, 128], [1, C]]
            )
            vb_bc = const.tile([128, C], fp32, tag="vbbc")
            nc.sync.dma_start(vb_bc[:], vb_bcast_ap)
            mc_sb = const.tile([128, 1], fp32, tag="mc")
            nc.vector.memset(mc_sb[:], -2.5)
            # pre-warm the exp/ln ACT table set during input DMA
            warm = const.tile([32, 1], fp32, tag="warm")
            nc.vector.memset(warm[:], 1.0)
            nc.scalar.activation(warm[:], warm[:], AF.Exp, scale=1.0)

            # ---------------- groupnorm ----------------
            stats = work.tile([128, 4, 2, 6], fp32, tag="stats")
            for j in range(4):
                for u in range(2):
                    nc.vector.bn_stats(
                        stats[:, j, u, :], x_sb[:, j, u * 512 : u * 512 + 512]
                    )
            mv = work.tile([128, 4, 2], fp32, tag="mv")
            for j in range(4):
                nc.vector.bn_aggr(mv[:, j, :], stats[:, j, :, :])
            ssq = work.tile([128, 4, 2], fp32, tag="ssq")
            nc.vector.tensor_copy(ssq[:, :, 0], mv[:, :, 0])
            nc.vector.tensor_tensor(ssq[:, :, 1], mv[:, :, 0], mv[:, :, 0], op=OP.mult)
            nc.vector.tensor_tensor(ssq[:, :, 1], ssq[:, :, 1], mv[:, :, 1], op=OP.add)
            # group stats [32, 2] = (mu_g, E[x^2]_g); G has 1/16 entries
            ps_g = psq.tile([32, 2], fp32, tag="q")
            for j in range(4):
                nc.tensor.matmul(
                    ps_g[:], G_sb[:, j, :], ssq[:, j, :], start=(j == 0), stop=(j == 3)
                )
            st2 = work.tile([32, 2], fp32, tag="st2")
            nc.vector.tensor_copy(st2[:, 0:1], ps_g[:, 0:1])
            var = work.tile([32, 1], fp32, tag="var")
            nc.vector.tensor_tensor(var[:], st2[:, 0:1], st2[:, 0:1], op=OP.mult)
            nc.vector.tensor_tensor(var[:], ps_g[:, 1:2], var[:], op=OP.subtract)
            eps_sb = const.tile([32, 1], fp32, tag="eps")
            nc.vector.memset(eps_sb[:], float(EPS))
            nc.scalar.activation(var[:], var[:], AF.Ln, bias=eps_sb[:], scale=1.0)
            nc.scalar.activation(st2[:, 1:2], var[:], AF.Exp, scale=-0.5)
            ps_bc = psq.tile([128, 4, 2], fp32, tag="q")
            for j in range(4):
                nc.tensor.matmul(
                    ps_bc[:, j, :], GT_sb[:, j, :], st2[:], start=True, stop=True
                )
            ab = work.tile([128, 4, 2], fp32, tag="ab")
            nc.vector.tensor_tensor(ab[:, :, 0], ps_bc[:, :, 1], nw_sb[:], op=OP.mult)
            nc.vector.tensor_tensor(ab[:, :, 1], ps_bc[:, :, 0], ab[:, :, 0], op=OP.mult)
            nc.vector.tensor_tensor(ab[:, :, 1], nb_sb[:], ab[:, :, 1], op=OP.subtract)

            xn_bf = work.tile([128, 4, N], bf16, tag="xn")
            for j in range(4):
                nc.gpsimd.tensor_scalar(
                    xn_bf[:, j, :],
                    x_sb[:, j, :],
                    ab[:, j, 0:1],
                    ab[:, j, 1:2],
                    op0=OP.mult,
                    op1=OP.add,
                )

            # ---------------- V^T = xn^T @ wV^T : [n, head, 65] ----------------
            # column 64 of each head's block is all-ones -> rowsum in AV row 64
            VT65 = work.tile([128, 8, 8, 66], bf16, tag="VT65")
            nc.vector.memset(VT65[:, :, :, 64:66], 1.0)

            def emit_v_chunks(mcs):
                for mc in mcs:
                    ps_v = psq.tile([128, 512], fp32, tag="q", name="ps_v")
                    for k in range(4):
                        nc.tensor.matmul(
                            ps_v[:],
                            xn_bf[:, k, mc * 128 : mc * 128 + 128],
                            w_bf[:, k, 2 * C : 3 * C],
                            start=(k == 0),
                            stop=(k == 3),
                        )
                    nc.vector.tensor_tensor(
                        VT65[:, mc, :, 0:64],
                        ps_v[:].rearrange("q (h d) -> q h d", h=8),
                        vb_bc[:].rearrange("q (h d) -> q h d", h=8),
                        op=OP.add,
                    )

            # ---------------- Q, K chunks (emitted interleaved) ----------------
            QK_bf = work.tile([128, 8, N], bf16, tag="QK")

            def emit_qk(oc):
                ps_qk = pss.tile([128, 1024], fp32, tag="s")
                for nu in range(2):
                    for k in range(4):
                        nc.tensor.matmul(
                            ps_qk[:, nu * 512 : nu * 512 + 512],
                            w_bf[:, k, oc * 128 : oc * 128 + 128],
                            xn_bf[:, k, nu * 512 : nu * 512 + 512],
                            start=(k == 0),
                            stop=(k == 3),
                        )
                nc.vector.tensor_scalar(
                    QK_bf[:, oc, :], ps_qk[:], qb_sb[:, oc : oc + 1], None, op0=OP.add
                )

            def emit_qk_small(oc):
                # mid-attention QK: psav bank, two half generations
                for nu in range(2):
                    ps_qk = psq.tile([128, 512], fp32, tag="q", name="ps_qk")
                    for k in range(4):
                        nc.tensor.matmul(
                            ps_qk[:],
                            w_bf[:, k, oc * 128 : oc * 128 + 128],
                            xn_bf[:, k, nu * 512 : nu * 512 + 512],
                            start=(k == 0),
                            stop=(k == 3),
                        )
                    nc.vector.tensor_scalar(
                        QK_bf[:, oc, nu * 512 : nu * 512 + 512],
                        ps_qk[:],
                        qb_sb[:, oc : oc + 1],
                        None,
                        op0=OP.add,
                    )

            att = work.tile([128, 4, N], fp32, tag="att")
            att_r = att.bitcast(fp32r)

            def s_mm(units, p, nu, s):
                for e in range(2):
                    nc.tensor.matmul(
                        units[:, e * 512 : e * 512 + 512],
                        QK_bf[e * 64 : e * 64 + 64, 4 + p, s * 128 : s * 128 + 128],
                        QK_bf[e * 64 : e * 64 + 64, p, nu * 512 : nu * 512 + 512],
                        start=True,
                        stop=True,
                        skip_group_check=True,
                    )

            def emit_pass(p, nu, fillers={}):
                """One nu-half of head-pair p: 8 s-steps of S -> exp -> AV."""
                av = [
                    psav.tile([128, 512], fp32, tag="av", name=f"av{e}")
                    for e in range(2)
                ]
                units = pss.tile([128, 1024], fp32, tag="s", name="ps_s")
                s_mm(units, p, nu, 0)
                for s in range(8):
                    P_bf = ppool.tile([128, 2, 512], bf16, tag="P")
                    nc.scalar.activation(
                        P_bf[:],
                        units[:].rearrange("q (h n) -> q h n", h=2),
                        AF.Exp,
                        scale=0.125,
                    )
                    if s < 7:
                        units = pss.tile([128, 1024], fp32, tag="s", name="ps_s")
                        s_mm(units, p, nu, s + 1)
                    for e in range(2):
                        nc.tensor.matmul(
                            av[e][0:65, :],
                            VT65[:, s, 2 * p + e, 0:65],
                            P_bf[:, e, :],
                            start=(s == 0),
                            stop=(s == 7),
                            skip_group_check=True,
                        )
                    if s in fillers:
                        fillers[s]()
                # rowsum rows -> partition 0, recip there, DMA roundtrip bcast
                rr_st = rpool.tile([128, 2, 512], fp32, tag="Rb", name="rrst")
                nc.scalar.copy(rr_st[0:1, 0, :], av[0][64:65, :])
                nc.vector.tensor_copy(rr_st[0:1, 1, :], av[1][64:65, :])
                rr_raw = rpool.tile([128, 2, 512], fp32, tag="Rb", name="rraw")
                nc.vector.reciprocal_approx_fast(
                    rr_raw[0:1, :, :].rearrange("q e n -> q (e n)"),
                    rr_st[0:1, :, :].rearrange("q e n -> q (e n)"),
                )
                nc.sync.dma_start(r8r_d.ap()[p, nu], rr_raw[0:1, :, :])
                for e in range(2):
                    nc.vector.tensor_copy(
                        att_r[e * 64 : e * 64 + 64, p, nu * 512 : nu * 512 + 512],
                        av[e][0:64, :],
                    )
                Rb = rpool.tile([128, 2, 512], fp32, tag="Rb", name="Rb")
                for e, eng in ((0, nc.sync), (1, nc.scalar)):
                    src_ap = r8r_d.ap()[p, nu, e]
                    bcast = bass_mod.AP(
                        tensor=src_ap.tensor,
                        offset=src_ap.offset,
                        ap=[[0, 64], [1, 512]],
                    )
                    eng.dma_start(Rb[e * 64 : e * 64 + 64, 0, :], bcast)
                nc.gpsimd.tensor_tensor(
                    att_r[:, p, nu * 512 : nu * 512 + 512],
                    att[:, p, nu * 512 : nu * 512 + 512],
                    Rb[:, 0, :],
                    op=OP.mult,
                )

            emit_qk(0)
            emit_qk(4)
            emit_v_chunks([0, 1, 2, 3, 4, 5])
            emit_pass(0, 0, {
                0: lambda: emit_v_chunks([6]),
                2: lambda: emit_v_chunks([7]),
                5: lambda: emit_qk_small(1),
            })
            emit_pass(0, 1, {2: lambda: emit_qk_small(5)})
            emit_pass(1, 0, {2: lambda: emit_qk_small(2)})
            emit_pass(1, 1, {2: lambda: emit_qk_small(6)})
            emit_pass(2, 0, {2: lambda: emit_qk_small(3)})
            emit_pass(2, 1, {2: lambda: emit_qk_small(7)})
            emit_pass(3, 0)
            emit_pass(3, 1)

            # ---------------- proj + residual ----------------
            for j in range(4):
                nc.vector.tensor_scalar(
                    x_sb[:, j, :], x_sb[:, j, :], pb_sb[:, j : j + 1], None, op0=OP.add
                )
            for oc in range(4):
                ps_o = pss.tile([128, 1024], fp32, tag="s")
                for nu in range(2):
                    for k in range(4):
                        nc.tensor.matmul(
                            ps_o[:, nu * 512 : nu * 512 + 512],
                            p_r[:, k, oc * 128 : oc * 128 + 128],
                            att_r[:, k, nu * 512 : nu * 512 + 512],
                            start=(k == 0),
                            stop=(k == 3),
                        )
                nc.vector.tensor_tensor(
                    x_sb[:, oc, :], ps_o[:], x_sb[:, oc, :], op=OP.add
                )
                _y8 = y_d.ap().rearrange("(j p) (u n) -> j u p n", p=128, n=512)
                for u in range(2):
                    [nc.sync, nc.scalar][u].dma_start(
                        _y8[oc, u], x_sb[:, oc, u * 512 : u * 512 + 512]
                    )

    nc.compile()
    return nc


def _get_nc(debug=False):
    if "nc" not in _cache:
        _cache["nc"] = _build_bass()
    return _cache["nc"]


def _host_inputs(x, norm_w, norm_b, qkv_w, qkv_b, proj_w, proj_b):
    import ml_dtypes

    x = np.asarray(x, dtype=np.float32).reshape(B, C, N)
    wqkvT = np.ascontiguousarray(
        np.asarray(qkv_w, dtype=np.float32).T.astype(ml_dtypes.bfloat16)
    )
    projT = np.ascontiguousarray(np.asarray(proj_w, dtype=np.float32).T)
    G = np.zeros((128, 4, 32), dtype=np.float32)
    GT = np.zeros((32, 4, 128), dtype=np.float32)
    for j in range(4):
        for p in range(128):
            g = 8 * j + p // 16
            G[p, j, g] = 1.0 / 16.0
            GT[g, j, p] = 1.0
    shared = {
        "wqkvT": wqkvT,
        "projT": projT,
        "qkv_b": np.asarray(qkv_b, dtype=np.float32),
        "proj_b": np.asarray(proj_b, dtype=np.float32),
        "norm_w": np.asarray(norm_w, dtype=np.float32),
        "norm_b": np.asarray(norm_b, dtype=np.float32),
        "Gmat": G,
        "GTmat": GT,
    }
    in_maps = [dict(shared, x=np.ascontiguousarray(x[i])) for i in range(B)]
    return in_maps


def kernel(x, norm_w, norm_b, qkv_w, qkv_b, proj_w, proj_b, _trace=False):
    from concourse import bass_utils

    nc = _get_nc()
    in_maps = _host_inputs(x, norm_w, norm_b, qkv_w, qkv_b, proj_w, proj_b)
    res = bass_utils.run_bass_kernel_spmd(
        nc, in_maps, core_ids=list(range(B)), trace=_trace
    )
    out = np.stack([res.results[i]["y"] for i in range(B)])
    _cache["last_result"] = res
    return out.reshape(B, C, 32, 32)
